# revision 16
# baseline (speedup 1.0000x reference)
"""TRN2 Bass/Tile kernel for nn_AttentionMixer (B=4, S=2048, D=1024, H=16).

Sharding (8 cores, no collectives):
  core c -> batch b = c // 2, head-group g = c % 2 (heads 8g..8g+7).
  Each core computes its 8 heads of attention for its batch plus the
  partial output projection (its 512 rows of Wout). The host sums the
  two partials per batch (the "all-reduce" of the tensor-parallel split).

v2: global-pipeline scheduler. The PE is in-order, so ScalarE (exp) can
only look as far ahead as the PE issue order allows. The emitter keeps
three decoupled streams and interleaves them into one PE issue order:
  - QK stream: per (pair, qc, kt) unit: 2 half-array score matmuls
    (row-alternating, which the PE overlaps ~1.7x) into a 2-slot PSUM
    ring + one [128,1024] exp on ScalarE into a deep (EXPB) bf16 ring.
  - AV stream: consumes exp tiles (P-stationary, 8 x N=65 matmuls per
    unit) into a 2-bank PSUM accumulator; lags the QK stream by an
    elastic amount so ScalarE never starves while the PE chews bursty
    weave work (the V projection must complete inside the first qc).
  - weave: projection groups (Q/K/V JIT per consumer deadline), y
    transposes, output-projection chunks, paced by due-dates and a
    fill-credit so the PE never idles while ScalarE is the local
    bottleneck.

Empirical PE costs (measured on HW): K=64 row-alternating pair ~608cyc
per kt; N=65 AV matmul ~103cyc; N=512 full-row matmul ~650cyc. PE busy
~290us/core, ScalarE ~255us -> PE-bound; target is PE ~100% occupancy.

attn_mask is all-ones by construction (spec fill=ones), so masking is a
no-op and is skipped.
"""

import numpy as np
from contextlib import ExitStack

import concourse.bass as bass
import concourse.bacc as bacc
import concourse.tile as tile
from concourse import mybir
from concourse.bass_utils import run_bass_kernel_spmd

F32 = mybir.dt.float32
MMDT = mybir.dt.bfloat16
AF = mybir.ActivationFunctionType
ALU = mybir.AluOpType

B, S, D, H = 4, 2048, 1024, 16
HD = 64          # head dim
HPC = 8          # heads per core
DH = HPC * HD    # 512: Wout rows per core
NDT = D // 128   # 8 d-tiles (contraction tiles for projections)
NKT = S // 128   # 16 key-token tiles
NQC = S // 512   # 4 query chunks of 512
NCORES = 8
NPAIR = 4        # head pairs per core
NUNIT = NPAIR * NQC * NKT  # 256 (pair, qc, kt) units

EXPB = 24        # exp ring depth (bf16 [128,2,512] tiles, 2KB/partition each)
LAG = 6          # nominal AV lag behind QK
FILL = 1450      # weave fill credit per step (PE cycles)


def decode(k):
    pair, r = divmod(k, NQC * NKT)
    qc, kt = divmod(r, NKT)
    return pair, qc, kt


class Emitter:
    def __init__(self, tc, nc, ctx, xT, wqkv, wout, ident, out, variant="full"):
        self.tc, self.nc = tc, nc
        self.out = out
        self.variant = variant

        self.p_x = ctx.enter_context(tc.tile_pool(name="x", bufs=1))
        self.p_w = ctx.enter_context(tc.tile_pool(name="w", bufs=1))
        self.p_v = ctx.enter_context(tc.tile_pool(name="v", bufs=1))
        self.p_q = ctx.enter_context(tc.tile_pool(name="q", bufs=2))
        self.p_k = ctx.enter_context(tc.tile_pool(name="k", bufs=2))
        self.p_exp = ctx.enter_context(tc.tile_pool(name="exp", bufs=EXPB))
        self.p_yn = ctx.enter_context(tc.tile_pool(name="yn", bufs=2))
        self.p_ysb = ctx.enter_context(tc.tile_pool(name="ysb", bufs=2))
        self.p_yt = ctx.enter_context(tc.tile_pool(name="yt", bufs=1))
        self.p_small = ctx.enter_context(tc.tile_pool(name="small", bufs=3))
        self.p_out = ctx.enter_context(tc.tile_pool(name="o", bufs=3))
        self.p_opart = ctx.enter_context(tc.tile_pool(name="opart", bufs=32))
        self.p_sc = ctx.enter_context(
            tc.tile_pool(name="sc", bufs=2, space=bass.MemorySpace.PSUM))
        self.p_y = ctx.enter_context(
            tc.tile_pool(name="py", bufs=1, space=bass.MemorySpace.PSUM))
        self.p_pq = ctx.enter_context(
            tc.tile_pool(name="pq", bufs=2, space=bass.MemorySpace.PSUM))

        self.wq_re = wqkv.rearrange("(dt p) n -> p dt n", p=128)
        self.wo_re = wout.rearrange("(dj p) n -> p dj n", p=128)
        self.xT = xT
        self.ident_dram = ident

        # scheduler state
        self.qk_ptr = 0
        self.av_ptr = 0
        self.exp_fifo = []
        self.weave = []          # items: dict(fn, cost, gate_qk, due_qk, due_av)
        # start deeply negative: the prologue is DMA-bound, so only
        # due-dated items may be pulled in until the pipeline is flowing
        self.fill_credit = -30000.0
        self.av_cool = 0
        self.o_parts = {}

    # ---- persistent loads ------------------------------------------------
    def loads(self):
        nc = self.nc
        # Parallel DMA queues: x chunks on the sync (SP) queue, Q/K weights
        # on the DVE queue, V weights + ident on the gpsimd queue, so the
        # first K projection (wk + chunk0) can start ~3us in.
        self.xt = [
            self.p_x.tile([128, S], MMDT, tag=f"xt{dt}", name=f"xt{dt}")
            for dt in range(NDT)
        ]
        for tcn in range(NQC):
            for dt in range(NDT):
                nc.sync.dma_start(
                    self.xt[dt][:, tcn * 512:(tcn + 1) * 512],
                    self.xT[dt * 128:(dt + 1) * 128, tcn * 512:(tcn + 1) * 512],
                )
        self.wk = self.p_w.tile([128, NDT, 512], MMDT, tag="wk", name="wk")
        nc.scalar.dma_start(self.wk[:], self.wq_re[:, :, 512:1024])
        self.wq = self.p_w.tile([128, NDT, 512], MMDT, tag="wqq", name="wq")
        nc.scalar.dma_start(self.wq[:], self.wq_re[:, :, 0:512])
        self.wv = self.p_w.tile([128, NDT, 512], MMDT, tag="wv", name="wv")
        nc.scalar.dma_start(self.wv[:], self.wq_re[:, :, 1024:1536])

        self.ident_sb = self.p_small.tile([128, 128], MMDT, tag="ident", name="ident")
        nc.scalar.dma_start(self.ident_sb[:], self.ident_dram[:, :])

        # V resident: [128 tok, kt, head, 65] with col 64 = 1.0 (Z ones).
        # Memset on gpsimd so the DVE queue stays clear for proj evacs.
        self.v_all = self.p_v.tile([128, NKT, HPC, HD + 1], MMDT, tag="vall",
                                   name="v_all")
        nc.gpsimd.memset(self.v_all[:], 1.0)

        # y psum accumulator: per (head, qsub): cols 0..64 = [y(64) | Z]
        self.y_ps = self.p_y.tile([128, 2, NQC, 128], F32, tag="py", name="y_ps")

        self.wo_sb = []
        self.yt = [
            self.p_yt.tile([128, S], MMDT, tag=f"yt{j}", name=f"yt{j}")
            for j in range(NPAIR)
        ]
        self.qk_tiles = []
        for j in range(NPAIR):
            qt = self.p_q.tile([128, S], MMDT, tag="qt", name=f"qt{j}")
            kt_t = self.p_k.tile([128, S], MMDT, tag="kt", name=f"kt{j}")
            self.qk_tiles.append((qt, kt_t))

    def load_wo(self):
        nc = self.nc
        for wi in range(2):
            t = self.p_w.tile([128, 2, D], MMDT, tag=f"wo{wi}", name=f"wo{wi}")
            nc.gpsimd.dma_start(t[:], self.wo_re[:, wi * 2:(wi + 1) * 2, :])
            self.wo_sb.append(t)

    # ---- work-item emitters ---------------------------------------------
    def emit_proj_group(self, pair, dst_kind, tcn):
        """Project Q or K pair rows for token chunk tcn: 8 accumulating
        N=512 matmuls + DVE evac to the bf16 pair tile."""
        nc = self.nc
        qt, kt_t = self.qk_tiles[pair]
        dst = qt if dst_kind == "q" else kt_t
        w = self.wq if dst_kind == "q" else self.wk
        sub = pair  # 128-row slice of the 512 q/k rows for this pair
        ps = self.p_pq.tile([128, 512], F32, tag="pq", name="ps_proj")
        for dt in range(NDT):
            nc.tensor.matmul(
                ps[:],
                w[:, dt, sub * 128:(sub + 1) * 128],
                self.xt[dt][:, tcn * 512:(tcn + 1) * 512],
                start=(dt == 0),
                stop=(dt == NDT - 1),
            )
        nc.vector.tensor_copy(dst[:, tcn * 512:(tcn + 1) * 512], ps[:])

    def emit_v_group(self, tt):
        """V for token tile tt, all 8 heads: x-stationary, weights moving
        (two N=256 halves), scattered into the 65-stride v_all layout."""
        nc = self.nc
        ps = self.p_pq.tile([128, 512], F32, tag="pq", name="ps_v")
        for half in range(2):
            for dt in range(NDT):
                nc.tensor.matmul(
                    ps[:, half * 256:(half + 1) * 256],
                    self.xt[dt][:, tt * 128:(tt + 1) * 128],
                    self.wv[:, dt, half * 256:(half + 1) * 256],
                    start=(dt == 0),
                    stop=(dt == NDT - 1),
                )
        nc.vector.tensor_copy(
            self.v_all[:, tt, :, 0:HD],
            ps[:].rearrange("p (h d) -> p h d", h=HPC),
        )

    def emit_qk_unit(self):
        nc = self.nc
        k = self.qk_ptr
        pair, qc, kt = decode(k)
        qt, kt_t = self.qk_tiles[pair]
        sc = self.p_sc.tile([128, 2, 512], F32, tag="sc", name="sc_t")
        for hh in range(2):
            bp = 64 * hh
            nc.tensor.matmul(
                sc[:, hh, :],
                kt_t[bp:bp + 64, kt * 128:(kt + 1) * 128],
                qt[bp:bp + 64, qc * 512:(qc + 1) * 512],
                start=True,
                stop=True,
            )
        expt = self.p_exp.tile([128, 2, 512], MMDT, tag="exp", name="exp_t")
        if self.variant == "noexp":
            # timing variant: tiny activation just to allocate the tile
            nc.scalar.activation(expt[:, 0, 0:8], sc[:, 0, 0:8], AF.Exp,
                                 scale=0.125)
        else:
            nc.scalar.activation(expt[:], sc[:], AF.Exp, scale=0.125)
        self.exp_fifo.append(expt)
        self.qk_ptr += 1

    def emit_av_unit(self):
        nc = self.nc
        k = self.av_ptr
        pair, qc, kt = decode(k)
        expt = self.exp_fifo.pop(0)
        if self.variant == "noav":
            if kt == NKT - 1:
                # tiny matmul to allocate y_ps for the epilogue readers
                nc.tensor.matmul(self.y_ps[:, 0, 0, 0:HD + 1],
                                 expt[:, 0, 0:128],
                                 self.v_all[:, kt, 2 * pair, :],
                                 start=True, stop=True)
            self.av_ptr += 1
            if kt == NKT - 1:
                self.emit_epilogue(pair, qc)
                self.av_cool = 2
            return
        for hh in range(2):
            for qs in range(4):
                nc.tensor.matmul(
                    self.y_ps[:, hh, qs, 0:HD + 1],
                    expt[:, hh, qs * 128:(qs + 1) * 128],
                    self.v_all[:, kt, 2 * pair + hh, :],
                    start=(kt == 0 and qs == 0),
                    stop=(kt == NKT - 1 and qs == 3),
                )
        self.av_ptr += 1
        if kt == NKT - 1:
            self.emit_epilogue(pair, qc)
            self.av_cool = 2

    def emit_epilogue(self, pair, qc):
        """Normalize on DVE now (frees y_ps for the next qc after it
        drains); transposes + evac + out chunks go into the weave."""
        nc = self.nc
        # single fast PSUM evac: the next qc's AV (start=True) only has to
        # wait for this copy, not the whole normalize chain
        ysb = self.p_ysb.tile([128, 2, NQC, HD + 1], F32, tag="ysb", name="ysb")
        nc.vector.tensor_copy(ysb[:], self.y_ps[:, :, :, 0:HD + 1])
        zr = self.p_small.tile([128, 2, NQC, 1], F32, tag="zr", name="zr")
        nc.vector.reciprocal(zr[:], ysb[:, :, :, HD:HD + 1])
        # yn laid out [128, qs, hh, d] so each qs slice is a contiguous
        # [128, 128] stationary for the combined transpose matmul
        yn = self.p_yn.tile([128, NQC, 2, HD], MMDT, tag="yn", name="yn")
        for hh in range(2):
            for qs in range(4):
                nc.vector.tensor_scalar(
                    yn[:, qs, hh, :],
                    ysb[:, hh, qs, 0:HD],
                    zr[:, hh, qs, :],
                    None,
                    ALU.mult,
                )

        def transp(half):
            # one matmul per qs block: stationary = both heads' yn
            # ([128, 2, 64] -> lhsT free 128 = out partitions, rows 0-63
            # head A dims, 64-127 head B), moving = identity
            p_tr = self.p_pq.tile([128, 256], F32, tag="pq", name="p_tr")
            for i in range(2):
                qs = half * 2 + i
                nc.tensor.matmul(
                    p_tr[:, i * 128:(i + 1) * 128],
                    yn[:, qs, :, :],
                    self.ident_sb[:],
                    start=True,
                    stop=True,
                )
            nc.vector.tensor_copy(
                self.yt[pair][:, (qc * 4 + half * 2) * 128:
                              (qc * 4 + half * 2 + 2) * 128],
                p_tr[:],
            )

        av_now = self.av_ptr
        for half in range(2):
            self.push(lambda h=half: transp(h), cost=2 * 166 + 60,
                      due_av=av_now + 1 + half)
        if pair == 1:
            # stage A: partial out chunks over pairs 0-1, woven into pair 2
            for qt_i in range(4 * qc, 4 * qc + 4):
                for oc in range(2):
                    self.push(lambda q=qt_i, o=oc: self.emit_out_partial(q, o),
                              cost=2 * 627 + 60,
                              gate_qk=2 * NQC * NKT,
                              due_qk=2 * NQC * NKT + qc * 14
                              + (qt_i % 4) * 3 + oc + 2)
        if pair == NPAIR - 1:
            for qt_i in range(4 * qc, 4 * qc + 4):
                for oc in range(2):
                    self.push(lambda q=qt_i, o=oc: self.emit_out_chunk(q, o),
                              cost=2 * 627 + 60,
                              due_av=av_now + 3 + (qt_i % 4) * 4 + oc * 2)

    def emit_out_partial(self, qt_i, oc):
        nc = self.nc
        ps = self.p_pq.tile([128, 512], F32, tag="pq", name="ps_op")
        for dj in range(2):
            nc.tensor.matmul(
                ps[:],
                self.yt[dj][:, qt_i * 128:(qt_i + 1) * 128],
                self.wo_sb[dj // 2][:, dj % 2, oc * 512:(oc + 1) * 512],
                start=(dj == 0),
                stop=(dj == 1),
            )
        op = self.p_opart.tile([128, 512], MMDT, tag="op", name="o_part")
        nc.vector.tensor_copy(op[:], ps[:])
        self.o_parts[(qt_i, oc)] = op

    def emit_out_chunk(self, qt_i, oc):
        nc = self.nc
        ps = self.p_pq.tile([128, 512], F32, tag="pq", name="ps_o")
        for dj in range(2, 4):
            nc.tensor.matmul(
                ps[:],
                self.yt[dj][:, qt_i * 128:(qt_i + 1) * 128],
                self.wo_sb[dj // 2][:, dj % 2, oc * 512:(oc + 1) * 512],
                start=(dj == 2),
                stop=(dj == 3),
            )
        o_stage = self.p_out.tile([128, 512], F32, tag="o", name="o_stage")
        nc.vector.tensor_add(o_stage[:], ps[:], self.o_parts.pop((qt_i, oc))[:])
        nc.sync.dma_start(
            self.out[qt_i * 128:(qt_i + 1) * 128, oc * 512:(oc + 1) * 512],
            o_stage[:],
        )

    # ---- scheduler -------------------------------------------------------
    def push(self, fn, cost, gate_qk=0, due_qk=1 << 30, due_av=1 << 30):
        self.weave.append(
            {"fn": fn, "cost": cost, "gate": gate_qk, "dq": due_qk, "da": due_av})

    def pump_due(self):
        # emit due items (scan the whole list; it stays short)
        i = 0
        while i < len(self.weave):
            it = self.weave[i]
            if (it["dq"] <= self.qk_ptr or it["da"] <= self.av_ptr) \
                    and it["gate"] <= self.qk_ptr:
                self.weave.pop(i)
                it["fn"]()
                self.fill_credit -= it["cost"]
            else:
                i += 1

    def pump_fill(self):
        while self.fill_credit > 0 and self.weave:
            it = self.weave[0]
            if it["gate"] > self.qk_ptr:
                break
            self.weave.pop(0)
            it["fn"]()
            self.fill_credit -= it["cost"]

    def build_weave(self):
        """Static projection work with JIT due-dates. The first K/Q groups
        (pair 0, tcn 0) are emitted directly in run() before the QK stream."""
        # pair 0 remaining projections
        for tcn in range(1, NQC):
            self.push(lambda t=tcn: self.emit_proj_group(0, "k", t),
                      cost=8 * 627 + 60, due_qk=max(0, 4 * tcn - 2))
        for tcn in range(1, NQC):
            self.push(lambda t=tcn: self.emit_proj_group(0, "q", t),
                      cost=8 * 627 + 60, due_qk=max(0, 16 * tcn - 3))
        # V groups: due just before their first AV consumer
        for tt in range(NKT):
            self.push(lambda t=tt: self.emit_v_group(t),
                      cost=16 * 294 + 60, due_av=tt)
        # pairs 1..3 projections: gated one pair ahead, due JIT
        for pair in range(1, NPAIR):
            base = pair * NQC * NKT
            gate = (pair - 1) * NQC * NKT
            for tcn in range(NQC):
                self.push(lambda p=pair, t=tcn: self.emit_proj_group(p, "k", t),
                          cost=8 * 627 + 60, gate_qk=gate,
                          due_qk=base + max(0, 4 * tcn - 2) - 2)
            for tcn in range(NQC):
                self.push(lambda p=pair, t=tcn: self.emit_proj_group(p, "q", t),
                          cost=8 * 627 + 60, gate_qk=gate,
                          due_qk=base + max(0, 16 * tcn - 3) - 2)
        # Wout load: cheap DMA, before pair 3 epilogues need it
        self.push(self.load_wo, cost=10, gate_qk=NQC * NKT,
                  due_qk=NQC * NKT + 40)
        # keep due order sorted-ish so the head-window scan finds due items
        self.weave.sort(key=lambda it: min(it["dq"], it["da"] + 40))

    def run(self):
        self.loads()
        self.build_weave()
        # prologue: first K and Q projections for pair 0 token-chunk 0
        self.emit_proj_group(0, "k", 0)
        self.emit_proj_group(0, "q", 0)
        while self.qk_ptr < NUNIT or self.av_ptr < NUNIT:
            # Greedy: fill the exp ring (deep ScalarE backlog that absorbs
            # PE bursts); consume one AV when the ring is full or the QK
            # stream is exhausted. A short cooldown after each epilogue
            # keeps the DVE normalize off the PE critical path.
            lag = self.qk_ptr - self.av_ptr
            if self.qk_ptr < NUNIT and lag < EXPB - 2:
                self.emit_qk_unit()
            elif self.av_ptr < NUNIT and (self.av_cool == 0 or lag >= EXPB - 1):
                self.emit_av_unit()
            if self.av_cool > 0:
                self.av_cool -= 1
            self.fill_credit = min(self.fill_credit, 0) + FILL
            self.pump_due()
            self.pump_fill()
        # drain remaining weave (final epilogue transposes + out chunks)
        while self.weave:
            it = self.weave.pop(0)
            it["fn"]()


def _emit(tc, nc, xT, wqkv, wout, ident, out, loop_n=1, variant="full"):
    ctx = ExitStack()
    with ctx:
        em = Emitter(tc, nc, ctx, xT, wqkv, wout, ident, out, variant=variant)
        if loop_n > 1:
            with tc.For_i(0, loop_n, 1):
                em.run()
        else:
            em.run()


def build_program(loop_n=1, variant="full"):
    nc = bacc.Bacc("TRN2", target_bir_lowering=False, debug=False)
    xT = nc.dram_tensor("xT", [D, S], MMDT, kind="ExternalInput").ap()
    wqkv = nc.dram_tensor("wqkv", [D, 3 * DH], MMDT, kind="ExternalInput").ap()
    wout = nc.dram_tensor("wout", [DH, D], MMDT, kind="ExternalInput").ap()
    ident = nc.dram_tensor("ident", [128, 128], MMDT, kind="ExternalInput").ap()
    out = nc.dram_tensor("out", [S, D], F32, kind="ExternalOutput").ap()
    with tile.TileContext(nc) as tc:
        _emit(tc, nc, xT, wqkv, wout, ident, out, loop_n=loop_n, variant=variant)
    nc.compile()
    return nc


_NC = None


def _get_nc():
    global _NC
    if _NC is None:
        _NC = build_program()
    return _NC


def _bf16():
    import ml_dtypes
    return ml_dtypes.bfloat16


def shard_inputs(x, Wqkv, Wout):
    ident = np.eye(128, dtype=_bf16())
    ins = []
    for c in range(NCORES):
        b, g = c // 2, c % 2
        xT_c = np.ascontiguousarray(x[b].T).astype(_bf16())
        wqkv_c = np.ascontiguousarray(
            np.concatenate(
                [Wqkv[:, comp * D + g * DH:comp * D + (g + 1) * DH] for comp in range(3)],
                axis=1,
            )
        ).astype(_bf16())
        wout_c = np.ascontiguousarray(Wout[g * DH:(g + 1) * DH, :]).astype(_bf16())
        ins.append({"xT": xT_c, "wqkv": wqkv_c, "wout": wout_c, "ident": ident})
    return ins


class PjrtRunner:
    """Persistent jitted SPMD runner (one trace/compile/load, many calls) —
    mirrors bass2jax.run_bass_via_pjrt's multi-core path."""

    def __init__(self, nc):
        import jax
        from jax.sharding import Mesh, PartitionSpec
        from jax.experimental.shard_map import shard_map
        from concourse import bass2jax
        from concourse.bass2jax import _bass_exec_p, partition_id_tensor, mybir as _mb

        bass2jax.install_neuronx_cc_hook()
        self.nc = nc
        partition_name = (
            nc.partition_id_tensor.name if nc.partition_id_tensor else None
        )
        in_names, out_names, out_avals, zero_outs = [], [], [], []
        for alloc in nc.m.functions[0].allocations:
            if not isinstance(alloc, _mb.MemoryLocationSet):
                continue
            name = alloc.memorylocations[0].name
            if alloc.kind == "ExternalInput":
                if name != partition_name:
                    in_names.append(name)
            elif alloc.kind == "ExternalOutput":
                shape = tuple(alloc.tensor_shape)
                dtype = _mb.dt.np(alloc.dtype)
                out_names.append(name)
                out_avals.append(jax.core.ShapedArray(shape, dtype))
                zero_outs.append(np.zeros(shape, dtype))
        self.in_names = list(in_names)
        self.out_names = out_names
        self.out_avals = out_avals
        self.zero_outs = zero_outs
        n_params = len(in_names)
        all_in = in_names + out_names
        if partition_name is not None:
            all_in = all_in + [partition_name]

        def _body(*args):
            operands = list(args)
            if partition_name is not None:
                operands.append(partition_id_tensor())
            return tuple(
                _bass_exec_p.bind(
                    *operands,
                    out_avals=tuple(out_avals),
                    in_names=tuple(all_in),
                    out_names=tuple(out_names),
                    lowering_input_output_aliases=(),
                    sim_require_finite=True,
                    sim_require_nnan=True,
                    nc=nc,
                )
            )

        devices = jax.devices()[:NCORES]
        mesh = Mesh(np.asarray(devices), ("core",))
        n_outs = len(out_names)
        self._fn = jax.jit(
            shard_map(
                _body,
                mesh=mesh,
                in_specs=(PartitionSpec("core"),) * (n_params + n_outs),
                out_specs=(PartitionSpec("core"),) * n_outs,
                check_rep=False,
            ),
            keep_unused=True,
        )

    def __call__(self, in_maps):
        import jax
        concat_in = [
            np.concatenate([np.asarray(m[name]) for m in in_maps], axis=0)
            for name in self.in_names
        ]
        concat_zeros = [
            np.zeros((NCORES * z.shape[0], *z.shape[1:]), z.dtype)
            for z in self.zero_outs
        ]
        out_arrs = self._fn(*concat_in, *concat_zeros)
        out_arrs = jax.block_until_ready(out_arrs)
        return [
            {
                name: np.asarray(out_arrs[i]).reshape(
                    NCORES, *self.out_avals[i].shape
                )[c]
                for i, name in enumerate(self.out_names)
            }
            for c in range(NCORES)
        ]


_RUNNER = None


def _get_runner():
    global _RUNNER
    if _RUNNER is None:
        _RUNNER = PjrtRunner(_get_nc())
    return _RUNNER


def kernel(x, attn_mask, Wqkv, Wout):
    x = np.asarray(x)
    Wqkv = np.asarray(Wqkv)
    Wout = np.asarray(Wout)
    ins = shard_inputs(x, Wqkv, Wout)
    res = run_bass_kernel_spmd(_get_nc(), ins, core_ids=list(range(NCORES)))
    out = np.empty((B, S, D), np.float32)
    for b in range(B):
        out[b] = res.results[2 * b]["out"] + res.results[2 * b + 1]["out"]
    return out


# revision 17
# speedup vs baseline: 1.1009x; 1.1009x over previous
"""TRN2 Bass/Tile kernel for nn_AttentionMixer (B=4, S=2048, D=1024, H=16).

Sharding (8 cores, no collectives):
  core c -> batch b = c // 2, head-group g = c % 2 (heads 8g..8g+7).
  Each core computes its 8 heads of attention for its batch plus the
  partial output projection (its 512 rows of Wout). The host sums the
  two partials per batch (the "all-reduce" of the tensor-parallel split).

v2: global-pipeline scheduler. The PE is in-order, so ScalarE (exp) can
only look as far ahead as the PE issue order allows. The emitter keeps
three decoupled streams and interleaves them into one PE issue order:
  - QK stream: per (pair, qc, kt) unit: 2 half-array score matmuls
    (row-alternating, which the PE overlaps ~1.7x) into a 2-slot PSUM
    ring + one [128,1024] exp on ScalarE into a deep (EXPB) bf16 ring.
  - AV stream: consumes exp tiles (P-stationary, 8 x N=65 matmuls per
    unit) into a 2-bank PSUM accumulator; lags the QK stream by an
    elastic amount so ScalarE never starves while the PE chews bursty
    weave work (the V projection must complete inside the first qc).
  - weave: projection groups (Q/K/V JIT per consumer deadline), y
    transposes, output-projection chunks, paced by due-dates and a
    fill-credit so the PE never idles while ScalarE is the local
    bottleneck.

Empirical PE costs (measured on HW): K=64 row-alternating pair ~608cyc
per kt; N=65 AV matmul ~103cyc; N=512 full-row matmul ~650cyc. PE busy
~290us/core, ScalarE ~255us -> PE-bound; target is PE ~100% occupancy.

attn_mask is all-ones by construction (spec fill=ones), so masking is a
no-op and is skipped.
"""

import numpy as np
from contextlib import ExitStack

import concourse.bass as bass
import concourse.bacc as bacc
import concourse.tile as tile
from concourse import mybir
from concourse.bass_utils import run_bass_kernel_spmd

F32 = mybir.dt.float32
MMDT = mybir.dt.bfloat16
AF = mybir.ActivationFunctionType
ALU = mybir.AluOpType

B, S, D, H = 4, 2048, 1024, 16
HD = 64          # head dim
HPC = 8          # heads per core
DH = HPC * HD    # 512: Wout rows per core
NDT = D // 128   # 8 d-tiles (contraction tiles for projections)
NKT = S // 128   # 16 key-token tiles
NQC = S // 512   # 4 query chunks of 512
NCORES = 8
NPAIR = 4        # head pairs per core
NUNIT = NPAIR * NQC * NKT  # 256 (pair, qc, kt) units

EXPB = 30        # exp ring depth (bf16 [128,2,512] tiles, 2KB/partition each)
LAG = 6          # nominal AV lag behind QK
FILL = 1450      # weave fill credit per step (PE cycles)


def decode(k):
    pair, r = divmod(k, NQC * NKT)
    qc, kt = divmod(r, NKT)
    return pair, qc, kt


class Emitter:
    def __init__(self, tc, nc, ctx, xT, wqkv, wout, ident, out, variant="full"):
        self.tc, self.nc = tc, nc
        self.out = out
        self.variant = variant

        self.p_x = ctx.enter_context(tc.tile_pool(name="x", bufs=1))
        self.p_w = ctx.enter_context(tc.tile_pool(name="w", bufs=1))
        self.p_v = ctx.enter_context(tc.tile_pool(name="v", bufs=1))
        self.p_q = ctx.enter_context(tc.tile_pool(name="q", bufs=2))
        self.p_k = ctx.enter_context(tc.tile_pool(name="k", bufs=2))
        self.p_exp = ctx.enter_context(tc.tile_pool(name="exp", bufs=EXPB))
        self.p_yn = ctx.enter_context(tc.tile_pool(name="yn", bufs=2))
        self.p_ysb = ctx.enter_context(tc.tile_pool(name="ysb", bufs=2))
        self.p_yt = ctx.enter_context(tc.tile_pool(name="yt", bufs=1))
        self.p_small = ctx.enter_context(tc.tile_pool(name="small", bufs=3))
        self.p_out = ctx.enter_context(tc.tile_pool(name="o", bufs=4))
        self.p_sc = ctx.enter_context(
            tc.tile_pool(name="sc", bufs=2, space=bass.MemorySpace.PSUM))
        self.p_y = ctx.enter_context(
            tc.tile_pool(name="py", bufs=1, space=bass.MemorySpace.PSUM))
        self.p_pq = ctx.enter_context(
            tc.tile_pool(name="pq", bufs=2, space=bass.MemorySpace.PSUM))

        self.wq_re = wqkv.rearrange("(dt p) n -> p dt n", p=128)
        self.wo_re = wout.rearrange("(dj p) n -> p dj n", p=128)
        self.xT = xT
        self.ident_dram = ident

        # scheduler state
        self.qk_ptr = 0
        self.av_ptr = 0
        self.exp_fifo = []
        self.weave = []          # items: dict(fn, cost, gate_qk, due_qk, due_av)
        # start deeply negative: the prologue is DMA-bound, so only
        # due-dated items may be pulled in until the pipeline is flowing
        self.fill_credit = -30000.0
        self.av_cool = 0
        self.o_parts = {}

    # ---- persistent loads ------------------------------------------------
    def loads(self):
        nc = self.nc
        # Parallel DMA queues: x chunks on the sync (SP) queue, Q/K weights
        # on the DVE queue, V weights + ident on the gpsimd queue, so the
        # first K projection (wk + chunk0) can start ~3us in.
        self.xt = [
            self.p_x.tile([128, S], MMDT, tag=f"xt{dt}", name=f"xt{dt}")
            for dt in range(NDT)
        ]
        for tcn in range(NQC):
            for dt in range(NDT):
                nc.sync.dma_start(
                    self.xt[dt][:, tcn * 512:(tcn + 1) * 512],
                    self.xT[dt * 128:(dt + 1) * 128, tcn * 512:(tcn + 1) * 512],
                )
        self.wk = self.p_w.tile([128, NDT, 512], MMDT, tag="wk", name="wk")
        nc.scalar.dma_start(self.wk[:], self.wq_re[:, :, 512:1024])
        self.wq = self.p_w.tile([128, NDT, 512], MMDT, tag="wqq", name="wq")
        nc.scalar.dma_start(self.wq[:], self.wq_re[:, :, 0:512])
        self.wv = self.p_w.tile([128, NDT, 512], MMDT, tag="wv", name="wv")
        nc.scalar.dma_start(self.wv[:], self.wq_re[:, :, 1024:1536])

        self.ident_sb = self.p_small.tile([128, 128], MMDT, tag="ident", name="ident")
        nc.scalar.dma_start(self.ident_sb[:], self.ident_dram[:, :])

        # V resident: [128 tok, kt, head, 65] with col 64 = 1.0 (Z ones).
        # Memset on gpsimd so the DVE queue stays clear for proj evacs.
        self.v_all = self.p_v.tile([128, NKT, HPC, HD + 1], MMDT, tag="vall",
                                   name="v_all")
        nc.gpsimd.memset(self.v_all[:], 1.0)

        # y psum accumulator: per (head, qsub): cols 0..64 = [y(64) | Z]
        self.y_ps = self.p_y.tile([128, 2, NQC, 128], F32, tag="py", name="y_ps")

        self.wo_sb = []
        self.yt = [
            self.p_yt.tile([128, S], MMDT, tag=f"yt{j}", name=f"yt{j}")
            for j in range(NPAIR)
        ]
        self.qk_tiles = []
        for j in range(NPAIR):
            qt = self.p_q.tile([128, S], MMDT, tag="qt", name=f"qt{j}")
            kt_t = self.p_k.tile([128, S], MMDT, tag="kt", name=f"kt{j}")
            self.qk_tiles.append((qt, kt_t))

    def load_wo(self):
        nc = self.nc
        for wi in range(2):
            t = self.p_w.tile([128, 2, D], MMDT, tag=f"wo{wi}", name=f"wo{wi}")
            nc.gpsimd.dma_start(t[:], self.wo_re[:, wi * 2:(wi + 1) * 2, :])
            self.wo_sb.append(t)

    # ---- work-item emitters ---------------------------------------------
    def emit_proj_group(self, pair, dst_kind, tcn):
        """Project Q or K pair rows for token chunk tcn: 8 accumulating
        N=512 matmuls + DVE evac to the bf16 pair tile."""
        nc = self.nc
        qt, kt_t = self.qk_tiles[pair]
        dst = qt if dst_kind == "q" else kt_t
        w = self.wq if dst_kind == "q" else self.wk
        sub = pair  # 128-row slice of the 512 q/k rows for this pair
        ps = self.p_pq.tile([128, 512], F32, tag="pq", name="ps_proj")
        for dt in range(NDT):
            nc.tensor.matmul(
                ps[:],
                w[:, dt, sub * 128:(sub + 1) * 128],
                self.xt[dt][:, tcn * 512:(tcn + 1) * 512],
                start=(dt == 0),
                stop=(dt == NDT - 1),
            )
        nc.vector.tensor_copy(dst[:, tcn * 512:(tcn + 1) * 512], ps[:])

    def emit_v_group(self, tt):
        """V for token tile tt, all 8 heads: x-stationary, weights moving
        (two N=256 halves), scattered into the 65-stride v_all layout."""
        nc = self.nc
        ps = self.p_pq.tile([128, 512], F32, tag="pq", name="ps_v")
        for half in range(2):
            for dt in range(NDT):
                nc.tensor.matmul(
                    ps[:, half * 256:(half + 1) * 256],
                    self.xt[dt][:, tt * 128:(tt + 1) * 128],
                    self.wv[:, dt, half * 256:(half + 1) * 256],
                    start=(dt == 0),
                    stop=(dt == NDT - 1),
                )
        nc.vector.tensor_copy(
            self.v_all[:, tt, :, 0:HD],
            ps[:].rearrange("p (h d) -> p h d", h=HPC),
        )

    def emit_qk_unit(self):
        nc = self.nc
        k = self.qk_ptr
        pair, qc, kt = decode(k)
        qt, kt_t = self.qk_tiles[pair]
        sc = self.p_sc.tile([128, 2, 512], F32, tag="sc", name="sc_t")
        for hh in range(2):
            bp = 64 * hh
            nc.tensor.matmul(
                sc[:, hh, :],
                kt_t[bp:bp + 64, kt * 128:(kt + 1) * 128],
                qt[bp:bp + 64, qc * 512:(qc + 1) * 512],
                start=True,
                stop=True,
            )
        expt = self.p_exp.tile([128, 2, 512], MMDT, tag="exp", name="exp_t")
        if self.variant == "noexp":
            # timing variant: tiny activation just to allocate the tile
            nc.scalar.activation(expt[:, 0, 0:8], sc[:, 0, 0:8], AF.Exp,
                                 scale=0.125)
        else:
            nc.scalar.activation(expt[:], sc[:], AF.Exp, scale=0.125)
        self.exp_fifo.append(expt)
        self.qk_ptr += 1

    def emit_av_unit(self):
        nc = self.nc
        k = self.av_ptr
        pair, qc, kt = decode(k)
        expt = self.exp_fifo.pop(0)
        if self.variant == "noav":
            if kt == NKT - 1:
                # tiny matmul to allocate y_ps for the epilogue readers
                nc.tensor.matmul(self.y_ps[:, 0, 0, 0:HD + 1],
                                 expt[:, 0, 0:128],
                                 self.v_all[:, kt, 2 * pair, :],
                                 start=True, stop=True)
            self.av_ptr += 1
            if kt == NKT - 1:
                self.emit_epilogue(pair, qc)
                self.av_cool = 2
            return
        for hh in range(2):
            for qs in range(4):
                nc.tensor.matmul(
                    self.y_ps[:, hh, qs, 0:HD + 1],
                    expt[:, hh, qs * 128:(qs + 1) * 128],
                    self.v_all[:, kt, 2 * pair + hh, :],
                    start=(kt == 0 and qs == 0),
                    stop=(kt == NKT - 1 and qs == 3),
                )
        self.av_ptr += 1
        if kt == NKT - 1:
            self.emit_epilogue(pair, qc)
            self.av_cool = 2

    def emit_epilogue(self, pair, qc):
        """Normalize on DVE now (frees y_ps for the next qc after it
        drains); transposes + evac + out chunks go into the weave."""
        nc = self.nc
        # single fast PSUM evac: the next qc's AV (start=True) only has to
        # wait for this copy, not the whole normalize chain
        ysb = self.p_ysb.tile([128, 2, NQC, HD + 1], F32, tag="ysb", name="ysb")
        nc.vector.tensor_copy(ysb[:], self.y_ps[:, :, :, 0:HD + 1])
        zr = self.p_small.tile([128, 2, NQC, 1], F32, tag="zr", name="zr")
        nc.vector.reciprocal(zr[:], ysb[:, :, :, HD:HD + 1])
        # yn laid out [128, qs, hh, d] so each qs slice is a contiguous
        # [128, 128] stationary for the combined transpose matmul
        yn = self.p_yn.tile([128, NQC, 2, HD], MMDT, tag="yn", name="yn")
        for hh in range(2):
            for qs in range(4):
                nc.vector.tensor_scalar(
                    yn[:, qs, hh, :],
                    ysb[:, hh, qs, 0:HD],
                    zr[:, hh, qs, :],
                    None,
                    ALU.mult,
                )

        def transp(half):
            # one matmul per qs block: stationary = both heads' yn
            # ([128, 2, 64] -> lhsT free 128 = out partitions, rows 0-63
            # head A dims, 64-127 head B), moving = identity
            p_tr = self.p_pq.tile([128, 256], F32, tag="pq", name="p_tr")
            for i in range(2):
                qs = half * 2 + i
                nc.tensor.matmul(
                    p_tr[:, i * 128:(i + 1) * 128],
                    yn[:, qs, :, :],
                    self.ident_sb[:],
                    start=True,
                    stop=True,
                )
            nc.vector.tensor_copy(
                self.yt[pair][:, (qc * 4 + half * 2) * 128:
                              (qc * 4 + half * 2 + 2) * 128],
                p_tr[:],
            )

        av_now = self.av_ptr
        for half in range(2):
            self.push(lambda h=half: transp(h), cost=2 * 166 + 60,
                      due_av=av_now + 1 + half)
        if pair == NPAIR - 1:
            for qt_i in range(4 * qc, 4 * qc + 4):
                for oc in range(2):
                    self.push(lambda q=qt_i, o=oc: self.emit_out_chunk(q, o),
                              cost=4 * 627 + 60,
                              due_av=av_now + 3 + (qt_i % 4) * 4 + oc * 2)

    def emit_out_chunk(self, qt_i, oc):
        nc = self.nc
        ps = self.p_pq.tile([128, 512], F32, tag="pq", name="ps_o")
        for dj in range(4):
            nc.tensor.matmul(
                ps[:],
                self.yt[dj][:, qt_i * 128:(qt_i + 1) * 128],
                self.wo_sb[dj // 2][:, dj % 2, oc * 512:(oc + 1) * 512],
                start=(dj == 0),
                stop=(dj == 3),
            )
        o_stage = self.p_out.tile([128, 512], F32, tag="o", name="o_stage")
        nc.vector.tensor_copy(o_stage[:], ps[:])
        nc.sync.dma_start(
            self.out[qt_i * 128:(qt_i + 1) * 128, oc * 512:(oc + 1) * 512],
            o_stage[:],
        )

    # ---- scheduler -------------------------------------------------------
    def push(self, fn, cost, gate_qk=0, due_qk=1 << 30, due_av=1 << 30):
        self.weave.append(
            {"fn": fn, "cost": cost, "gate": gate_qk, "dq": due_qk, "da": due_av})

    def pump_due(self):
        # emit due items (scan the whole list; it stays short)
        i = 0
        while i < len(self.weave):
            it = self.weave[i]
            if (it["dq"] <= self.qk_ptr or it["da"] <= self.av_ptr) \
                    and it["gate"] <= self.qk_ptr:
                self.weave.pop(i)
                it["fn"]()
                self.fill_credit -= it["cost"]
            else:
                i += 1

    def pump_fill(self):
        while self.fill_credit > 0 and self.weave:
            it = self.weave[0]
            if it["gate"] > self.qk_ptr:
                break
            self.weave.pop(0)
            it["fn"]()
            self.fill_credit -= it["cost"]

    def build_weave(self):
        """Static projection work with JIT due-dates. The first K/Q groups
        (pair 0, tcn 0) are emitted directly in run() before the QK stream."""
        # pair 0 remaining projections
        for tcn in range(1, NQC):
            self.push(lambda t=tcn: self.emit_proj_group(0, "k", t),
                      cost=8 * 627 + 60, due_qk=max(0, 4 * tcn - 2))
        for tcn in range(1, NQC):
            self.push(lambda t=tcn: self.emit_proj_group(0, "q", t),
                      cost=8 * 627 + 60, due_qk=max(0, 16 * tcn - 3))
        # V groups: due just before their first AV consumer
        for tt in range(NKT):
            self.push(lambda t=tt: self.emit_v_group(t),
                      cost=16 * 294 + 60, due_av=tt)
        # pairs 1..3 projections: gated one pair ahead, due JIT
        for pair in range(1, NPAIR):
            base = pair * NQC * NKT
            gate = (pair - 1) * NQC * NKT
            for tcn in range(NQC):
                self.push(lambda p=pair, t=tcn: self.emit_proj_group(p, "k", t),
                          cost=8 * 627 + 60, gate_qk=gate,
                          due_qk=base + max(0, 4 * tcn - 2) - 2)
            for tcn in range(NQC):
                self.push(lambda p=pair, t=tcn: self.emit_proj_group(p, "q", t),
                          cost=8 * 627 + 60, gate_qk=gate,
                          due_qk=base + max(0, 16 * tcn - 3) - 2)
        # Wout load: cheap DMA, before pair 3 epilogues need it
        self.push(self.load_wo, cost=10, gate_qk=NQC * NKT,
                  due_qk=NQC * NKT + 40)
        # keep due order sorted-ish so the head-window scan finds due items
        self.weave.sort(key=lambda it: min(it["dq"], it["da"] + 40))

    def run(self):
        self.loads()
        self.build_weave()
        # prologue: first K and Q projections for pair 0 token-chunk 0
        self.emit_proj_group(0, "k", 0)
        self.emit_proj_group(0, "q", 0)
        while self.qk_ptr < NUNIT or self.av_ptr < NUNIT:
            # Greedy: fill the exp ring (deep ScalarE backlog that absorbs
            # PE bursts); consume one AV when the ring is full or the QK
            # stream is exhausted. A short cooldown after each epilogue
            # keeps the DVE normalize off the PE critical path.
            lag = self.qk_ptr - self.av_ptr
            if self.qk_ptr < NUNIT and lag < EXPB - 2:
                self.emit_qk_unit()
            elif self.av_ptr < NUNIT and (self.av_cool == 0 or lag >= EXPB - 1):
                self.emit_av_unit()
            if self.av_cool > 0:
                self.av_cool -= 1
            self.fill_credit = min(self.fill_credit, 0) + FILL
            self.pump_due()
            self.pump_fill()
        # drain remaining weave (final epilogue transposes + out chunks)
        while self.weave:
            it = self.weave.pop(0)
            it["fn"]()


def _emit(tc, nc, xT, wqkv, wout, ident, out, loop_n=1, variant="full"):
    ctx = ExitStack()
    with ctx:
        em = Emitter(tc, nc, ctx, xT, wqkv, wout, ident, out, variant=variant)
        if loop_n > 1:
            with tc.For_i(0, loop_n, 1):
                em.run()
        else:
            em.run()


def build_program(loop_n=1, variant="full"):
    nc = bacc.Bacc("TRN2", target_bir_lowering=False, debug=False)
    xT = nc.dram_tensor("xT", [D, S], MMDT, kind="ExternalInput").ap()
    wqkv = nc.dram_tensor("wqkv", [D, 3 * DH], MMDT, kind="ExternalInput").ap()
    wout = nc.dram_tensor("wout", [DH, D], MMDT, kind="ExternalInput").ap()
    ident = nc.dram_tensor("ident", [128, 128], MMDT, kind="ExternalInput").ap()
    out = nc.dram_tensor("out", [S, D], F32, kind="ExternalOutput").ap()
    with tile.TileContext(nc) as tc:
        _emit(tc, nc, xT, wqkv, wout, ident, out, loop_n=loop_n, variant=variant)
    nc.compile()
    return nc


_NC = None


def _get_nc():
    global _NC
    if _NC is None:
        _NC = build_program()
    return _NC


def _bf16():
    import ml_dtypes
    return ml_dtypes.bfloat16


def shard_inputs(x, Wqkv, Wout):
    ident = np.eye(128, dtype=_bf16())
    ins = []
    for c in range(NCORES):
        b, g = c // 2, c % 2
        xT_c = np.ascontiguousarray(x[b].T).astype(_bf16())
        wqkv_c = np.ascontiguousarray(
            np.concatenate(
                [Wqkv[:, comp * D + g * DH:comp * D + (g + 1) * DH] for comp in range(3)],
                axis=1,
            )
        ).astype(_bf16())
        wout_c = np.ascontiguousarray(Wout[g * DH:(g + 1) * DH, :]).astype(_bf16())
        ins.append({"xT": xT_c, "wqkv": wqkv_c, "wout": wout_c, "ident": ident})
    return ins


class PjrtRunner:
    """Persistent jitted SPMD runner (one trace/compile/load, many calls) —
    mirrors bass2jax.run_bass_via_pjrt's multi-core path."""

    def __init__(self, nc):
        import jax
        from jax.sharding import Mesh, PartitionSpec
        from jax.experimental.shard_map import shard_map
        from concourse import bass2jax
        from concourse.bass2jax import _bass_exec_p, partition_id_tensor, mybir as _mb

        bass2jax.install_neuronx_cc_hook()
        self.nc = nc
        partition_name = (
            nc.partition_id_tensor.name if nc.partition_id_tensor else None
        )
        in_names, out_names, out_avals, zero_outs = [], [], [], []
        for alloc in nc.m.functions[0].allocations:
            if not isinstance(alloc, _mb.MemoryLocationSet):
                continue
            name = alloc.memorylocations[0].name
            if alloc.kind == "ExternalInput":
                if name != partition_name:
                    in_names.append(name)
            elif alloc.kind == "ExternalOutput":
                shape = tuple(alloc.tensor_shape)
                dtype = _mb.dt.np(alloc.dtype)
                out_names.append(name)
                out_avals.append(jax.core.ShapedArray(shape, dtype))
                zero_outs.append(np.zeros(shape, dtype))
        self.in_names = list(in_names)
        self.out_names = out_names
        self.out_avals = out_avals
        self.zero_outs = zero_outs
        n_params = len(in_names)
        all_in = in_names + out_names
        if partition_name is not None:
            all_in = all_in + [partition_name]

        def _body(*args):
            operands = list(args)
            if partition_name is not None:
                operands.append(partition_id_tensor())
            return tuple(
                _bass_exec_p.bind(
                    *operands,
                    out_avals=tuple(out_avals),
                    in_names=tuple(all_in),
                    out_names=tuple(out_names),
                    lowering_input_output_aliases=(),
                    sim_require_finite=True,
                    sim_require_nnan=True,
                    nc=nc,
                )
            )

        devices = jax.devices()[:NCORES]
        mesh = Mesh(np.asarray(devices), ("core",))
        n_outs = len(out_names)
        self._fn = jax.jit(
            shard_map(
                _body,
                mesh=mesh,
                in_specs=(PartitionSpec("core"),) * (n_params + n_outs),
                out_specs=(PartitionSpec("core"),) * n_outs,
                check_rep=False,
            ),
            keep_unused=True,
        )

    def __call__(self, in_maps):
        import jax
        concat_in = [
            np.concatenate([np.asarray(m[name]) for m in in_maps], axis=0)
            for name in self.in_names
        ]
        concat_zeros = [
            np.zeros((NCORES * z.shape[0], *z.shape[1:]), z.dtype)
            for z in self.zero_outs
        ]
        out_arrs = self._fn(*concat_in, *concat_zeros)
        out_arrs = jax.block_until_ready(out_arrs)
        return [
            {
                name: np.asarray(out_arrs[i]).reshape(
                    NCORES, *self.out_avals[i].shape
                )[c]
                for i, name in enumerate(self.out_names)
            }
            for c in range(NCORES)
        ]


_RUNNER = None


def _get_runner():
    global _RUNNER
    if _RUNNER is None:
        _RUNNER = PjrtRunner(_get_nc())
    return _RUNNER


def kernel(x, attn_mask, Wqkv, Wout):
    x = np.asarray(x)
    Wqkv = np.asarray(Wqkv)
    Wout = np.asarray(Wout)
    ins = shard_inputs(x, Wqkv, Wout)
    res = run_bass_kernel_spmd(_get_nc(), ins, core_ids=list(range(NCORES)))
    out = np.empty((B, S, D), np.float32)
    for b in range(B):
        out[b] = res.results[2 * b]["out"] + res.results[2 * b + 1]["out"]
    return out
